# revision 17
# baseline (speedup 1.0000x reference)
"""CANet non-local attention block (sparse_attention) on 8 Trainium2 cores.

Math (per sample, reference.py):
    g     = maxpool2(conv1x1(x, g_w, g_b))        -> [CI, 2304]
    theta = conv1x1(x, theta_w, theta_b)          -> [CI, 9216]
    phi   = maxpool2(conv1x1(x, phi_w, phi_b))    -> [CI, 2304]
    f     = theta^T @ phi                         -> [9216, 2304]
    attn  = softmax(f, axis=-1)
    y     = attn @ g^T                            -> [9216, CI]
    out   = BN(conv1x1(y^T, W_w, W_b)) + x

Sharding: 8 cores = 4 samples x 2 query-halves. Each core convs/pools its
full sample (redundant with its pair core) and runs theta/attention for
its 4608-query half. The per-core xf is column-reordered so the core's
own half always sits at columns 0:4608 (kv order is softmax-invariant),
which lets one f16 xf tensor serve conv, theta AND the residual add.

Steady state is ACT(exp)-bound: 54 activations of [128,1536] at ~1.73us
issue rate. Everything else is arranged so ACT never waits:
  - phi bias: softmax-invariant -> dropped exactly.
  - g bias: folded into a host-side per-channel x-offset Delta solving
    (I + W'@g_w) Delta = b' + W'@g_b.
  - theta bias: DVE tensor_scalar_add on the PSUM->SBUF copy.
  - maxpool: DVE pool_max (w-pairs, fused f32 PSUM read + f16 cast) then
    GPSIMD tensor_max (h-pairs) - keeps the DVE under the ACT period.
  - softmax normalizer s from a ones-column in the g^T operand; 1/s via
    the single-op DVE reciprocal_approx_fast (~5x cheaper than the
    iterative reciprocal), partition-broadcast via a DRAM round trip
    (one block off the critical path; last block uses a K=1 PE matmul).
  - theta projections trickle through the steady loop as PE filler.
  - e-tile pool is 3 deep so exp(b+1,0) never waits on mm2(b-1) WAR.
  - ACT exp table pre-loaded by a dummy activation at t~7us (overlaps
    input DMA) instead of right before the first real exp.
  - PE warm-up matmuls flip the HAM clock gate to 8/8 while the first
    input DMAs are in flight; keepwarm ldweights hold it there.
"""

import sys
import types

if "/opt/trn_rl_repo" not in sys.path:
    sys.path.insert(0, "/opt/trn_rl_repo")

# antenv.axon_hooks is absent in this image, so trn_boot's NTFF hook install
# silently degrades. Provide the module and install the ctypes hook ourselves
# so run_bass_kernel_spmd(trace=True) can capture NTFF profiles.
try:
    import antenv

    if "antenv.axon_hooks" not in sys.modules:
        _m = types.ModuleType("antenv.axon_hooks")
        _hook_box = [None]

        def _set(h):
            _hook_box[0] = h

        def _get():
            return _hook_box[0]

        _m.set_axon_ntff_profile_hook = _set
        _m.get_axon_ntff_profile_hook = _get
        sys.modules["antenv.axon_hooks"] = _m
        antenv.axon_hooks = _m
        try:
            if "/root/.axon_site" not in sys.path:
                sys.path.insert(0, "/root/.axon_site")
            from trn_agent_boot.trn_boot import _ntff_profile_via_ctypes

            _hook_box[0] = _ntff_profile_via_ctypes("/opt/axon/libaxon_pjrt.so")
        except Exception:
            pass
except Exception:
    pass

import numpy as np

import concourse.bass as bass
import concourse.tile as tile
from concourse import mybir
from concourse.bass_utils import run_bass_kernel_spmd

F32 = mybir.dt.float32
F16 = mybir.dt.float16
BF16 = mybir.dt.bfloat16

B, C, CI, H, W = 4, 128, 64, 96, 96
N = H * W                   # 9216 queries per sample
NKV = (H // 2) * (W // 2)   # 2304 keys
NQH = N // 2                # 4608 queries per core
QB = 512                    # query block
NB = NQH // QB              # 9 blocks per core
KCH = NKV // 128            # 18 kv chunks of 128
GRP = 6                     # conv/exp groups of 1536 per block
EXP_SHIFT = -20.0

_PROGRAM = None


def _split_multi_waits(nc, max_waits=1):
    """walrus codegen in this container only accepts one sync-wait command
    per instruction; hoist extras onto injected same-engine NoOps."""
    n_new = 0
    for f in nc.m.functions:
        for bb in f.blocks:
            new_list = []
            for ins in bb.instructions:
                si = ins.sync_info
                w = list(si.on_wait) if si and si.on_wait else []
                if len(w) > max_waits:
                    extras, keep = w[:-max_waits], w[-max_waits:]
                    for ew in extras:
                        nop = mybir.InstNoOp(
                            name=f"I-ws{nc.next_id()}", ins=[], outs=[]
                        )
                        nop.engine = ins.engine
                        nop.sync_info = mybir.SyncInfo(on_wait=[ew], on_update=[])
                        new_list.append(nop)
                        n_new += 1
                    si.on_wait = keep
                    ins.sync_info = si
                new_list.append(ins)
            bb.instructions[:] = new_list
    return n_new


def _build_program():
    nc = bass.Bass("TRN2", target_bir_lowering=False, debug=False, num_devices=8)

    xf = nc.dram_tensor("xf", [C, N], F16, kind="ExternalInput")
    wgp = nc.dram_tensor("wgp", [C, 128], F16, kind="ExternalInput")
    wt2 = nc.dram_tensor("wt2", [C, 128], F16, kind="ExternalInput")
    wpc = nc.dram_tensor("wpc", [CI, C], BF16, kind="ExternalInput")
    bt2 = nc.dram_tensor("bt2", [C, 1], F32, kind="ExternalInput")
    id64 = nc.dram_tensor("id64", [CI, CI], F16, kind="ExternalInput")
    out = nc.dram_tensor("out", [C, NQH], F16, kind="ExternalOutput")
    rb_dram = nc.dram_tensor("rb_scratch", [NB, QB], F32)

    with tile.TileContext(nc) as tc:
        with (
            tc.tile_pool(name="const", bufs=1) as const,
            tc.tile_pool(name="main", bufs=1) as main,
            tc.tile_pool(name="mwp", bufs=3) as mwp,
            tc.tile_pool(name="small", bufs=4) as small,
            tc.tile_pool(name="yraw", bufs=3) as yraw,
            tc.tile_pool(name="ynrm", bufs=2) as ynrm,
            tc.tile_pool(name="otp", bufs=3) as otp,
            tc.tile_pool(name="brp", bufs=3) as brp,
            tc.tile_pool(name="epool", bufs=3) as epool,
            tc.tile_pool(name="fps", bufs=2, space="PSUM") as fps,
            tc.tile_pool(name="sps", bufs=2, space="PSUM") as sps,
        ):
            # Load the exp ACT table set immediately (no DMA deps): the
            # ~1.5us ACT_TABLE_LOAD overlaps the input DMA instead of
            # sitting right before the first real exp.
            warm_in = const.tile([1, 8], F32)
            nc.vector.memset(warm_in, 0.0)
            warm_out = const.tile([1, 8], F32)
            nc.scalar.activation(
                warm_out, warm_in, mybir.ActivationFunctionType.Exp
            )

            # PE warm-up: dummy matmuls with no DMA deps keep the PE busy
            # from the end of the NEFF preamble until the first conv's
            # input lands, priming the HAM clock gate toward 8/8.
            dmy_l = const.tile([C, 64], F16)
            nc.vector.memset(dmy_l, 0.0)
            dmy_r = const.tile([C, 256], F16)
            nc.vector.memset(dmy_r, 0.0)
            kw_ps = sps.tile([C, QB], F32, tag="sp", name="kw")
            for _ in range(5):
                nc.tensor.matmul(
                    kw_ps[0:CI, 0:256], lhsT=dmy_l, rhs=dmy_r, start=True,
                    stop=True,
                )

            # Input DMAs: xf split into 768-col pieces alternating between
            # the two HWDGE queues (SP and ACT) so transfers run in
            # parallel and conv group 0 starts ~2.5us sooner.
            xf_sb = main.tile([C, N], F16)
            wgp_sb = const.tile([C, 128], F16)
            nc.sync.dma_start(wgp_sb, wgp[:, :])
            wt2_sb = const.tile([C, 128], F16)
            nc.scalar.dma_start(wt2_sb, wt2[:, :])
            bt2_sb = const.tile([C, 1], F32)
            nc.scalar.dma_start(bt2_sb, bt2[:, :])
            id_sb = const.tile([CI, CI], F16)
            wpc_sb = const.tile([CI, C], BF16)
            for c in range(12):
                eng = nc.sync if c % 2 == 0 else nc.scalar
                eng.dma_start(
                    xf_sb[:, c * 768:(c + 1) * 768], xf[:, c * 768:(c + 1) * 768]
                )
                if c == 4:
                    nc.sync.dma_start(id_sb, id64[:, :])
                    nc.sync.dma_start(wpc_sb, wpc[:, :])
            shift_sb = const.tile([C, 1], F32)
            nc.vector.memset(shift_sb, EXP_SHIFT)
            ones_sb = const.tile([1, CI], F32)
            nc.vector.memset(ones_sb, 1.0)

            th_sb = main.tile([C, NQH], F16)     # theta on both partition halves
            P_sb = main.tile([C, NKV], F16)      # pooled [g(0:64); phi(64:128)]
            phi0 = main.tile([CI, NKV], F16)     # phi copy at base partition 0
            gt_sb = main.tile([C, KCH, CI + 1], BF16)  # g^T chunks + ones col
            nc.vector.memset(gt_sb, 1.0)         # bakes the ones column

            # ---------------- helpers ----------------
            def emit_theta(b, bias_engine="vector"):
                # All thetas run in the head (a steady-loop PSUM tile would
                # break the fps double-buffer parity that keeps mm1 one
                # group ahead of exp). Bias-adds go to ACT for the middle
                # thetas (slotted between head exps, which are pool-gated
                # and leave ACT idle time) and to DVE for the rest.
                tp = sps.tile([C, QB], F32, tag="sp", name=f"th{b}")
                nc.tensor.matmul(
                    tp,
                    lhsT=wt2_sb,
                    rhs=xf_sb[:, b * QB:(b + 1) * QB],
                    start=True,
                    stop=True,
                )
                if bias_engine == "scalar":
                    nc.scalar.add(th_sb[:, b * QB:(b + 1) * QB], tp, bt2_sb)
                else:
                    nc.vector.tensor_scalar_add(
                        th_sb[:, b * QB:(b + 1) * QB], tp, bt2_sb
                    )

            def emit_transpose_pair(p):
                c0 = 2 * p
                tp = sps.tile([C, 128], F16, tag="sp")
                nc.tensor.transpose(
                    tp[:, 0:CI], P_sb[0:CI, c0 * 128:(c0 + 1) * 128], id_sb
                )
                nc.tensor.transpose(
                    tp[:, CI:128], P_sb[0:CI, (c0 + 1) * 128:(c0 + 2) * 128],
                    id_sb,
                )
                v = tp[:, :].rearrange("p (a c) -> p a c", a=2, c=CI)
                nc.vector.tensor_copy(gt_sb[:, c0:c0 + 2, 0:CI], v)

            def emit_mm1_chunk(ft, b, j, u):
                if j % 2 == 0:
                    lhsT = phi0[:, j * 128:(j + 1) * 128]
                    rhs = th_sb[0:CI, b * QB:(b + 1) * QB]
                    pos = (0, 0)
                else:
                    lhsT = P_sb[CI:C, j * 128:(j + 1) * 128]
                    rhs = th_sb[CI:C, b * QB:(b + 1) * QB]
                    pos = (64, 0)
                nc.tensor.matmul(
                    ft[:, u * QB:(u + 1) * QB],
                    lhsT=lhsT,
                    rhs=rhs,
                    start=True,
                    stop=True,
                    tile_position=pos,
                )

            def emit_exp(e_t, ft, t):
                nc.scalar.activation(
                    e_t[:, t * 3 * QB:(t + 1) * 3 * QB],
                    ft,
                    mybir.ActivationFunctionType.Exp,
                    bias=shift_sb,
                )

            def emit_mm1(e_t, b, t):
                ft = fps.tile([C, 3 * QB], F32, tag="fp")
                for u in range(3):
                    emit_mm1_chunk(ft, b, 3 * t + u, u)
                emit_exp(e_t, ft, t)

            def emit_mm1_pair(e_t, b, k):
                # two exp-groups (2k, 2k+1) emitted so every PE slot is a
                # quadrant pair: (c0,c1), (c3,c4), then the two leftover
                # chunks c2 (even) and c5 (odd) pair with each other.
                # 3 paired slots instead of 2 pairs + 2 singles.
                fta = fps.tile([C, 3 * QB], F32, tag="fp")
                emit_mm1_chunk(fta, b, 6 * k, 0)
                emit_mm1_chunk(fta, b, 6 * k + 1, 1)
                ftb = fps.tile([C, 3 * QB], F32, tag="fp")
                emit_mm1_chunk(ftb, b, 6 * k + 3, 0)
                emit_mm1_chunk(ftb, b, 6 * k + 4, 1)
                emit_mm1_chunk(fta, b, 6 * k + 2, 2)
                emit_mm1_chunk(ftb, b, 6 * k + 5, 2)
                emit_exp(e_t, fta, 2 * k)
                emit_exp(e_t, ftb, 2 * k + 1)

            def emit_mm2(e_t, y_ps, chunks):
                for j in chunks:
                    nc.tensor.matmul(
                        y_ps[0:CI + 1, :],
                        lhsT=gt_sb[:, j, :],
                        rhs=e_t[:, j * QB:(j + 1) * QB],
                        start=(j == 0),
                        stop=(j == KCH - 1),
                        skip_group_check=True,
                    )

            y_tiles = {}
            br_tiles = {}

            def emit_block_end(b, y_ps):
                # one fused [65,512] PSUM->SBUF copy: y rows + the s row
                y_r = yraw.tile([CI + 1, QB], F32, tag="yr", name=f"yr{b}")
                nc.vector.tensor_copy(y_r, y_ps[0:CI + 1, :])
                y_tiles[b] = y_r
                r_t = small.tile([1, QB], F32, tag="r", name=f"r{b}")
                nc.vector.reciprocal(r_t, y_r[CI:CI + 1, :])
                if b + 1 < NB:
                    nc.sync.dma_start(rb_dram[b:b + 1, :], r_t)
                    br_sb = brp.tile([CI, QB], F32, tag="br", name=f"br{b}")
                    nc.sync.dma_start(
                        br_sb, rb_dram[b:b + 1, :].partition_broadcast(CI)
                    )
                    br_tiles[b] = br_sb
                else:
                    # tail: skip the DRAM round trip; PSUM is free by now, so
                    # broadcast r across partitions with a K=1 PE matmul
                    r_ps = sps.tile([CI, QB], F32, tag="sp", name="rps")
                    nc.tensor.matmul(
                        r_ps, lhsT=ones_sb, rhs=r_t, start=True, stop=True
                    )
                    br_tiles[b] = r_ps

            def emit_epilogue(b):
                y_n = ynrm.tile([CI, QB], BF16, tag="yn")
                nc.vector.tensor_mul(
                    y_n, y_tiles.pop(b)[0:CI, :], br_tiles.pop(b)
                )
                z_ps = sps.tile([C, QB], F32, tag="sp")
                nc.tensor.matmul(z_ps, lhsT=wpc_sb, rhs=y_n, start=True, stop=True)
                ot = otp.tile([C, QB], F16, tag="ot")
                nc.vector.tensor_add(
                    ot, z_ps, xf_sb[:, b * QB:(b + 1) * QB]
                )
                nc.sync.dma_start(out[:, b * QB:(b + 1) * QB], ot)

            # ---------------- head ----------------
            def emit_conv(t):
                ft = fps.tile([C, 3 * QB], F32, tag="fp", name=f"cft{t}")
                for u in range(3):
                    nc.tensor.matmul(
                        ft[:, u * QB:(u + 1) * QB],
                        lhsT=wgp_sb,
                        rhs=xf_sb[:, t * 1536 + u * QB:t * 1536 + (u + 1) * QB],
                        start=True,
                        stop=True,
                    )
                return ft

            def emit_pool(t, ft):
                # 2x2 maxpool from conv PSUM, split so the PSUM is read
                # exactly twice at half width: ScalarE casts the odd
                # columns (ACT is otherwise idle between head exps), DVE
                # maxes them against the even PSUM columns directly.
                vw = ft[:, :].rearrange("p (x a) -> p x a", a=2)
                godd = mwp.tile([C, 768], F16, tag="go", name=f"go{t}")
                nc.scalar.copy(godd, vw[:, :, 1])
                m1 = mwp.tile([C, 16, 48], F16, tag="mw", name=f"mw{t}")
                nc.vector.tensor_max(
                    m1[:, :, :].rearrange("p h w -> p (h w)"), vw[:, :, 0], godd
                )
                vh = m1[:, :, :].rearrange("p (h a) w -> p h a w", h=8, a=2)
                Pv = P_sb[:, t * 384:(t + 1) * 384].rearrange(
                    "p (h w) -> p h w", h=8, w=48
                )
                nc.vector.tensor_max(Pv, vh[:, :, 0, :], vh[:, :, 1, :])
                nc.vector.tensor_copy(
                    phi0[:, t * 384:(t + 1) * 384],
                    P_sb[CI:C, t * 384:(t + 1) * 384],
                )

            # Head alloc choreography: conv(1) is emitted FIRST so that in
            # the fps round-robin the conv stream owns slot A (chained by
            # pool(t+1) reads) and the mm1 stream owns slot B (chained by
            # the exps) - mm1(0,0) then waits on pool(0), not pool(1).
            e_tiles = {0: epool.tile([C, N], BF16, tag="e", name="e0")}
            tpair = 0
            ft1 = emit_conv(1)
            ft0 = emit_conv(0)
            emit_theta(0)
            emit_pool(0, ft0)
            emit_pool(1, ft1)
            fts = {}
            for t in range(GRP):
                if 2 <= t + 1 < GRP:
                    emit_pool(t + 1, fts.pop(t + 1))
                if t + 2 < GRP:
                    fts[t + 2] = emit_conv(t + 2)
                emit_mm1(e_tiles[0], 0, t)
                emit_theta(t + 1)
                if t == 4:
                    emit_theta(7)
                if t == 5:
                    emit_theta(8)
                # transposes for all kv-chunk pairs fully pooled so far
                while 2 * tpair + 1 <= 3 * t + 2:
                    emit_transpose_pair(tpair)
                    tpair += 1

            # ---------------- steady state: mm1/exp one block ahead of mm2,
            # epilogue one block behind ----------------
            for b in range(NB - 1):
                e_cur = e_tiles.pop(b)
                y_ps = sps.tile([C, QB], F32, tag="sp", name=f"y{b}")
                e_tiles[b + 1] = epool.tile(
                    [C, N], BF16, tag="e", name=f"e{b + 1}"
                )
                for k in range(3):
                    emit_mm1_pair(e_tiles[b + 1], b + 1, k)
                    emit_mm2(e_cur, y_ps, range(6 * k, 6 * k + 6))
                    if k == 1:
                        # keepwarm insurance against a HAM clock dip
                        nc.tensor.ldweights(e_tiles[b + 1][:, 0:128])
                    if k == 2 and b >= 1:
                        emit_epilogue(b - 1)
                emit_block_end(b, y_ps)
            # last block: its exps ran in the window above, so burst mm2(b8)
            # right here; the conv/mm1 PSUM pool is idle now - borrow a slot
            e_cur = e_tiles.pop(NB - 1)
            y_last = fps.tile([C, QB], F32, tag="fp", name="y8")
            for t in range(GRP):
                emit_mm2(e_cur, y_last, range(3 * t, 3 * t + 3))
            emit_epilogue(NB - 2)
            # inline last-block end: dummy matmuls keep the PE (and the HAM
            # clock) warm while the reciprocal runs, so the broadcast and
            # epilogue matmuls that follow run at full clock
            bl = NB - 1
            y_r = yraw.tile([CI + 1, QB], F32, tag="yr", name=f"yr{bl}")
            nc.vector.tensor_copy(y_r, y_last[0:CI + 1, :])
            y_tiles[bl] = y_r
            r_t = small.tile([1, QB], F32, tag="r", name=f"r{bl}")
            nc.vector.reciprocal(r_t, y_r[CI:CI + 1, :])
            for _ in range(24):
                nc.tensor.matmul(
                    y_last[CI:C, 0:256], lhsT=dmy_l, rhs=dmy_r,
                    start=True, stop=True,
                )
            r_ps = sps.tile([CI, QB], F32, tag="sp", name="rps")
            nc.tensor.matmul(r_ps, lhsT=ones_sb, rhs=r_t, start=True, stop=True)
            br_tiles[bl] = r_ps
            emit_epilogue(bl)

    _split_multi_waits(nc)
    return nc


def _get_program():
    global _PROGRAM
    if _PROGRAM is None:
        _PROGRAM = _build_program()
    return _PROGRAM


def _host_prep(x, g_w, g_b, theta_w, theta_b, phi_w, phi_b, W_w, W_b,
               bn_gamma, bn_beta, bn_mean, bn_var):
    f32 = np.float32

    import ml_dtypes

    inv = (bn_gamma / np.sqrt(bn_var + 1e-5)).astype(f32)
    Wp = (W_w * inv[:, None]).astype(f32)                  # [C, CI]
    bp = (W_b * inv + bn_beta - bn_mean * inv).astype(f32)  # [C]
    delta = np.linalg.solve(
        np.eye(C, dtype=np.float64) + (Wp @ g_w).astype(np.float64),
        (bp + Wp @ g_b).astype(np.float64),
    ).astype(f32)                                          # [C]
    bt_eff = (theta_b - theta_w @ delta).astype(f32)       # [CI]

    wgp = np.ascontiguousarray(np.concatenate([g_w, phi_w], 0).T.astype(np.float16))
    wt2 = np.ascontiguousarray(
        np.concatenate([theta_w.T, theta_w.T], 1).astype(np.float16)
    )
    wpc = np.ascontiguousarray(Wp.T.astype(ml_dtypes.bfloat16))
    bt2 = np.concatenate([bt_eff, bt_eff]).reshape(C, 1).astype(f32)
    id64 = np.eye(CI, dtype=np.float16)

    in_maps = []
    for core in range(8):
        s, h = core // 2, core % 2
        xs = (x[s] + delta[:, None, None]).astype(np.float16)
        # column-reorder: this core's query half first (kv order is
        # softmax-invariant, so conv/pool over the permuted sample is fine)
        xs_perm = np.concatenate(
            [xs[:, 48 * h:48 * (h + 1), :], xs[:, 48 * (1 - h):48 * (2 - h), :]],
            axis=1,
        )
        xf_full = np.ascontiguousarray(xs_perm.reshape(C, N))
        in_maps.append(
            {
                "xf": xf_full,
                "wgp": wgp,
                "wt2": wt2,
                "wpc": wpc,
                "bt2": bt2,
                "id64": id64,
            }
        )
    return in_maps


def run_cores(in_maps, trace=False):
    nc = _get_program()
    return run_bass_kernel_spmd(nc, in_maps, list(range(8)), trace=trace)


def kernel(**inputs) -> np.ndarray:
    in_maps = _host_prep(**inputs)
    res = run_cores(in_maps)
    out = np.empty((B, C, H, W), dtype=np.float32)
    for core in range(8):
        s, h = core // 2, core % 2
        out[s, :, 48 * h:48 * (h + 1), :] = res.results[core]["out"].reshape(
            C, 48, W
        )
    return out
